# revision 40
# baseline (speedup 1.0000x reference)
"""MoE model kernel for Trainium2 (8 NeuronCores, data-parallel over batch).

Reference computation (per token):
  router: 3-layer MLP (fp32) -> softmax -> top-2 gates (vals/2 scattered dense)
  experts: 8x (D->H1 relu, H1->H2 relu, H2->C) combined with gates
Outputs: (out [B, C] f32, probs [B, E] f32)

Sharding: batch B=16384 split across 8 cores (2048 tokens each); router and
all experts replicated on every core. Router runs in fp32 on the PE (top-2
selection needs fp32 accuracy: min prob gap between ranks 2/3 is ~1.5e-6).
Expert matmuls run in bf16 with fp32 PSUM accumulation (max err ~0.3% of
output scale). The dense gated combine matches the reference's math exactly
(gates are 0 for non-selected experts).
"""

import numpy as np
import ml_dtypes

import concourse.bacc as bacc
import concourse.bass as bass
import concourse.mybir as mybir
import concourse.tile as tile
from concourse.bass_utils import run_bass_kernel_spmd

F32 = mybir.dt.float32
BF16 = mybir.dt.bfloat16
AF = mybir.ActivationFunctionType
ALU = mybir.AluOpType
AX = mybir.AxisListType

B, D, E, C = 16384, 1024, 8, 50
RH = 512
H1, H2 = 2048, 1024
NCORES = 8
T = B // NCORES  # tokens per core
P = 128
NT = T // P      # 16 token tiles per core
CP = 64          # padded C for psum/acc tiles

# Sparse dispatch: per-expert slot capacities (multiples of 128), computed
# per input-set on the host (the routing distribution depends on the actual
# router weights/data, and the jax PRNG stream is backend-dependent). Tokens
# beyond capacity would be silently dropped, so capacities carry a >=25%
# + 128 margin over the max per-core count; compiled kernels are cached per
# capacity profile.
CAPS = (768, 1536, 256, 256, 768, 1152, 1152, 512)   # default profile
OFFS = tuple(int(np.sum(CAPS[:e])) for e in range(E))
NSLOT = int(np.sum(CAPS))
NPAD = T + P                        # outpad rows (last 128 = trash tokens)
CHUNK = 768                         # max slots per compute chunk

VARIANT = "sparse"

_built = {}


def set_caps(caps):
    global CAPS, OFFS, NSLOT
    CAPS = tuple(int(c) for c in caps)
    assert all(c % P == 0 and c > 0 for c in CAPS)
    OFFS = tuple(int(np.sum(CAPS[:e])) for e in range(E))
    NSLOT = int(np.sum(CAPS))


def compute_caps(inputs):
    """Host-side router pass -> per-expert capacities (max core count +25%+128,
    rounded up to 128)."""
    x = np.asarray(inputs["x"], np.float32)
    h = np.maximum(x @ np.asarray(inputs["rW1"], np.float32)
                   + np.asarray(inputs["rb1"], np.float32), 0)
    h = np.maximum(h @ np.asarray(inputs["rW2"], np.float32)
                   + np.asarray(inputs["rb2"], np.float32), 0)
    s = h @ np.asarray(inputs["rW3"], np.float32) + np.asarray(inputs["rb3"], np.float32)
    # top-2 selection only needs score order; softmax is monotone per row.
    thresh = np.sort(s, axis=1)[:, -2:-1]
    sel = s >= thresh
    counts = np.zeros((NCORES, E), np.int64)
    for c in range(NCORES):
        counts[c] = sel[c * T : (c + 1) * T].sum(0)
    worst = counts.max(0)
    caps = np.ceil((worst * 1.1 + 64) / P).astype(np.int64) * P
    caps = np.minimum(caps, T)
    return tuple(int(v) for v in caps)


def _router(nc, tc, io, pools, want_mask=False):
    """Router MLP in fp32 + softmax + top-2 gates.

    Writes probs into acc[:, :, 50:58] (packed output). Returns the gates
    tile [128, NT, E] f32 (token-major) in a long-lived pool (and the 0/1
    top-2 mask when want_mask).
    """
    gpool = pools["gates"]
    acc = pools["acc"]
    gates = gpool.tile([P, NT, E], F32, tag="gates", bufs=1)
    probs_sb = gpool.tile([P, NT, E], F32, tag="probs", bufs=1)
    mask = None
    if want_mask:
        mask = gpool.tile([P, NT, E], F32, tag="mask", bufs=1, name="mask")

    with (
        tc.tile_pool(name="rsb", bufs=1) as rsb,
        tc.tile_pool(name="rstream", bufs=3) as rstream,
        tc.tile_pool(name="rps", bufs=1, space="PSUM") as rps,
    ):
        # Router weights resident in SBUF (fp32, ~2.6MB total)
        rW1_sb = rsb.tile([P, D // P, RH], F32, tag="rW1")
        nc.sync.dma_start(rW1_sb[:], io["rW1"][:].rearrange("(k p) m -> p k m", p=P))
        rW2_sb = rsb.tile([P, RH // P, RH // 2], F32, tag="rW2")
        nc.sync.dma_start(rW2_sb[:], io["rW2"][:].rearrange("(k p) m -> p k m", p=P))
        rW3_sb = rsb.tile([P, (RH // 2) // P, E], F32, tag="rW3")
        nc.sync.dma_start(rW3_sb[:], io["rW3"][:].rearrange("(k p) m -> p k m", p=P))
        rb1_sb = rsb.tile([P, RH // P], F32, tag="rb1")
        nc.sync.dma_start(rb1_sb[:], io["rb1"][:].rearrange("(m p) -> p m", p=P))
        rb2_sb = rsb.tile([P, (RH // 2) // P], F32, tag="rb2")
        nc.sync.dma_start(rb2_sb[:], io["rb2"][:].rearrange("(m p) -> p m", p=P))
        rb3_sb = rsb.tile([1, E], F32, tag="rb3")
        nc.sync.dma_start(rb3_sb[:], io["rb3"][None, :])
        ones1 = rsb.tile([1, P], F32, tag="ones1")
        nc.vector.memset(ones1[:], 1.0)

        h1r = rsb.tile([P, RH // P, T], F32, tag="h1r")   # 4MB
        h2r = rsb.tile([P, (RH // 2) // P, T], F32, tag="h2r")  # 2MB

        NTOK = 512  # token tile for router layers
        xT_view = io["xT_f32"][:].rearrange("(k p) t -> p k t", p=P)

        # L1: h1r = relu(rW1.T @ xT + rb1)
        for n in range(T // NTOK):
            nsl = bass.ts(n, NTOK)
            xks = []
            for k in range(D // P):
                xk = rstream.tile([P, NTOK], F32, tag=f"xk{k % 2}", bufs=2)
                nc.sync.dma_start(xk[:], xT_view[:, k, nsl])
                xks.append(xk)
            psl1 = [
                rps.tile([P, NTOK], F32, tag=f"psl1_{m}", bufs=1, name=f"psl1_{m}")
                for m in range(RH // P)
            ]
            for k in range(D // P):
                for m in range(RH // P):
                    nc.tensor.matmul(
                        psl1[m][:],
                        rW1_sb[:, k, bass.ts(m, P)],
                        xks[k][:],
                        start=(k == 0),
                        stop=(k == D // P - 1),
                    )
            for m in range(RH // P):
                nc.scalar.activation(
                    h1r[:, m, nsl], psl1[m][:], AF.Relu, bias=rb1_sb[:, m : m + 1]
                )

        # L2: h2r = relu(rW2.T @ h1r + rb2)
        for n in range(T // NTOK):
            nsl = bass.ts(n, NTOK)
            psl2 = [
                rps.tile([P, NTOK], F32, tag=f"psl2_{m}", bufs=1, name=f"psl2_{m}")
                for m in range((RH // 2) // P)
            ]
            for k in range(RH // P):
                for m in range((RH // 2) // P):
                    nc.tensor.matmul(
                        psl2[m][:],
                        rW2_sb[:, k, bass.ts(m, P)],
                        h1r[:, k, nsl],
                        start=(k == 0),
                        stop=(k == RH // P - 1),
                    )
            for m in range((RH // 2) // P):
                nc.scalar.activation(
                    h2r[:, m, nsl], psl2[m][:], AF.Relu, bias=rb2_sb[:, m : m + 1]
                )

        # L3 (token-major): scores[t, e] = h2r.T @ rW3 + rb3
        scores = gpool.tile([P, NT, E], F32, tag="scores", bufs=1)
        for i in range(NT):
            ps3 = rps.tile([P, E], F32, tag="ps3", bufs=2)
            for k in range((RH // 2) // P):
                nc.tensor.matmul(
                    ps3[:],
                    h2r[:, k, bass.ts(i, P)],
                    rW3_sb[:, k, :],
                    start=(k == 0),
                    stop=False,
                )
            nc.tensor.matmul(ps3[:], ones1[:], rb3_sb[:], start=False, stop=True)
            nc.vector.tensor_copy(scores[:, i, :], ps3[:])

        # Softmax over E (per 8-wide segment)
        mx = rsb.tile([P, NT, 1], F32, tag="mx")
        nc.vector.tensor_reduce(mx[:], scores[:], axis=AX.X, op=ALU.max)
        xs = rsb.tile([P, NT, E], F32, tag="xs")
        nc.vector.tensor_tensor(
            xs[:], scores[:], mx[:].to_broadcast([P, NT, E]), op=ALU.subtract
        )
        ex = rsb.tile([P, NT, E], F32, tag="ex")
        nc.scalar.activation(ex[:], xs[:], AF.Exp)
        sm = rsb.tile([P, NT, 1], F32, tag="sm")
        nc.vector.tensor_reduce(sm[:], ex[:], axis=AX.X, op=ALU.add)
        rs = rsb.tile([P, NT, 1], F32, tag="rs")
        nc.vector.reciprocal(rs[:], sm[:])
        nc.vector.tensor_tensor(
            probs_sb[:], ex[:], rs[:].to_broadcast([P, NT, E]), op=ALU.mult
        )
        nc.vector.tensor_copy(acc[:, :, C : C + E], probs_sb[:])

        # Top-2 gates: gates = probs * (probs >= 2nd_max) / 2
        m1 = rsb.tile([P, NT, 1], F32, tag="m1")
        nc.vector.tensor_reduce(m1[:], probs_sb[:], axis=AX.X, op=ALU.max)
        lt = rsb.tile([P, NT, E], F32, tag="lt")
        nc.vector.tensor_tensor(
            lt[:], probs_sb[:], m1[:].to_broadcast([P, NT, E]), op=ALU.is_lt
        )
        pz = rsb.tile([P, NT, E], F32, tag="pz")
        nc.vector.tensor_tensor(pz[:], probs_sb[:], lt[:], op=ALU.mult)
        m2 = rsb.tile([P, NT, 1], F32, tag="m2")
        nc.vector.tensor_reduce(m2[:], pz[:], axis=AX.X, op=ALU.max)
        ge = mask if want_mask else rsb.tile([P, NT, E], F32, tag="ge")
        nc.vector.tensor_tensor(
            ge[:], probs_sb[:], m2[:].to_broadcast([P, NT, E]), op=ALU.is_ge
        )
        ph = rsb.tile([P, NT, E], F32, tag="ph")
        nc.vector.tensor_scalar_mul(ph[:], probs_sb[:], 0.5)
        nc.vector.tensor_tensor(gates[:], ph[:], ge[:], op=ALU.mult)

    if want_mask:
        return gates, mask
    return gates


def _experts(nc, tc, io, pools, gates):
    """Dense expert compute in bf16 with gated fp32 combine."""
    acc = pools["acc"]

    TH = 1024       # tokens per half
    NTOK = 512      # matmul free dim
    NH = T // TH    # 2 halves

    with (
        tc.tile_pool(name="esb", bufs=1) as esb,
        tc.tile_pool(name="ew", bufs=1) as ew,
        tc.tile_pool(name="eps", bufs=1, space="PSUM") as eps,
    ):
        xTb_sb = esb.tile([P, D // P, T], BF16, tag="xTb")  # 4MB resident
        nc.sync.dma_start(
            xTb_sb[:], io["xT_bf16"][:].rearrange("(k p) t -> p k t", p=P)
        )
        onesb = esb.tile([1, P], BF16, tag="onesb")
        nc.vector.memset(onesb[:], 1.0)
        h1b = esb.tile([P, H1 // P, TH], BF16, tag="h1b")  # 4MB
        h2b = esb.tile([P, H2 // P, TH], BF16, tag="h2b")  # 2MB

        for e in range(E):
            eb1_sb = ew.tile([P, H1 // P], F32, tag="eb1", bufs=2)
            nc.sync.dma_start(
                eb1_sb[:], io[f"eb1_{e}"][:].rearrange("(m p) -> p m", p=P)
            )
            eb2_sb = ew.tile([P, H2 // P], F32, tag="eb2", bufs=2)
            nc.sync.dma_start(
                eb2_sb[:], io[f"eb2_{e}"][:].rearrange("(m p) -> p m", p=P)
            )
            w3_sb = ew.tile([P, H2 // P, C], BF16, tag="w3", bufs=2)
            nc.sync.dma_start(
                w3_sb[:], io[f"eW3_{e}"][:].rearrange("(k p) m -> p k m", p=P)
            )
            b3_sb = ew.tile([1, C], BF16, tag="b3", bufs=2)
            nc.sync.dma_start(b3_sb[:], io[f"eb3b_{e}"][:])

            w1_view = io[f"eW1_{e}"][:].rearrange("(k p) m -> p k m", p=P)
            w2_view = io[f"eW2_{e}"][:].rearrange("(k p) m -> p k m", p=P)

            for h in range(NH):
                hsl = slice(h * TH, (h + 1) * TH)
                # L1: h1b = relu(W1.T @ x + b1), M=H1 in 4 strips of 512
                for mi in range(H1 // NTOK):
                    w1s = ew.tile([P, D // P, NTOK], BF16, tag="w1s", bufs=3)
                    nc.sync.dma_start(w1s[:], w1_view[:, :, bass.ts(mi, NTOK)])
                    for mm in range(NTOK // P):
                        m = mi * (NTOK // P) + mm
                        for n in range(TH // NTOK):
                            nsl = slice(h * TH + n * NTOK, h * TH + (n + 1) * NTOK)
                            ps = eps.tile([P, NTOK], F32, tag="l1ps", bufs=3)
                            for k in range(D // P):
                                nc.tensor.matmul(
                                    ps[:],
                                    w1s[:, k, bass.ts(mm, P)],
                                    xTb_sb[:, k, nsl],
                                    start=(k == 0),
                                    stop=(k == D // P - 1),
                                )
                            nc.scalar.activation(
                                h1b[:, m, bass.ts(n, NTOK)],
                                ps[:],
                                AF.Relu,
                                bias=eb1_sb[:, m : m + 1],
                            )
                # L2: h2b = relu(W2.T @ h1b + b2), M=H2 in 2 strips of 512
                for mi in range(H2 // NTOK):
                    w2s = ew.tile([P, H1 // P, NTOK], BF16, tag="w2s", bufs=2)
                    nc.sync.dma_start(w2s[:], w2_view[:, :, bass.ts(mi, NTOK)])
                    for mm in range(NTOK // P):
                        m = mi * (NTOK // P) + mm
                        for n in range(TH // NTOK):
                            ps = eps.tile([P, NTOK], F32, tag="l2ps", bufs=3)
                            for k in range(H1 // P):
                                nc.tensor.matmul(
                                    ps[:],
                                    w2s[:, k, bass.ts(mm, P)],
                                    h1b[:, k, bass.ts(n, NTOK)],
                                    start=(k == 0),
                                    stop=(k == H1 // P - 1),
                                )
                            nc.scalar.activation(
                                h2b[:, m, bass.ts(n, NTOK)],
                                ps[:],
                                AF.Relu,
                                bias=eb2_sb[:, m : m + 1],
                            )
                # L3 token-major + gated combine
                for i in range(TH // P):
                    it = h * (TH // P) + i
                    ps3 = eps.tile([P, CP], F32, tag="l3ps", bufs=2)
                    for k in range(H2 // P):
                        nc.tensor.matmul(
                            ps3[:, :C],
                            h2b[:, k, bass.ts(i, P)],
                            w3_sb[:, k, :],
                            start=(k == 0),
                            stop=False,
                        )
                    nc.tensor.matmul(
                        ps3[:, :C], onesb[:], b3_sb[:], start=False, stop=True
                    )
                    tmp = ew.tile([P, CP], F32, tag="tmp", bufs=4)
                    nc.scalar.activation(
                        tmp[:, :C], ps3[:, :C], AF.Copy,
                        scale=gates[:, it, e : e + 1],
                    )
                    nc.vector.tensor_add(
                        acc[:, it, :C], acc[:, it, :C], tmp[:, :C]
                    )

        nc.sync.dma_start(io["outbuf"][:].rearrange("(i p) c -> p i c", p=P), acc[:])


def _dispatch(nc, tc, io, pools, gates, mask):
    """Build per-expert token slot lists from the top-2 mask.

    Slot position of token t in expert e = OFFS[e] + (# tokens < t routed to
    e), computed with triangular-matrix prefix-sum matmuls. Token ids and
    gate values are scatter-added into the idxlist DRAM table (init -1, add
    t+1 / 1+gate), then read back as gather/scatter index lists.

    Returns (gidx16 [128, NSLOT/16] int16 clamped >=0 for gathers,
             sidx16 [128, NSLOT/16] int16 with trash-token padding for the
             output scatter, gsl [128, NSLOT/128, 1] f32 per-slot gates).
    """
    gpool = pools["gates"]
    gidx16 = gpool.tile([P, NSLOT // 16], mybir.dt.int16, tag="gidx16", bufs=1)
    sidx16 = gpool.tile([P, NSLOT // 16], mybir.dt.int16, tag="sidx16", bufs=1)
    gsl = gpool.tile([P, NSLOT // P, 1], F32, tag="gsl", bufs=1)

    with (
        tc.tile_pool(name="dsb", bufs=1) as dsb,
        tc.tile_pool(name="dps", bufs=1, space="PSUM") as dps,
    ):
        tri128 = dsb.tile([P, P], F32, tag="tri128")
        nc.sync.dma_start(tri128[:], io["tri128"][:])
        tri16s = dsb.tile([16, 16], F32, tag="tri16s")
        nc.sync.dma_start(tri16s[:], io["tri16s"][:])
        idconst = dsb.tile([P, P], F32, tag="idconst")
        nc.sync.dma_start(idconst[:], io["idconst"][:])
        capoff = dsb.tile([1, E], F32, tag="capoff")
        nc.sync.dma_start(capoff[:], io["capoff"][:])
        trashv = dsb.tile([P, 1, 1], F32, tag="trashv")
        nc.sync.dma_start(trashv[:], io["trashv"][:])
        trash16 = dsb.tile([P, 1], F32, tag="trash16")
        nc.sync.dma_start(trash16[:], io["trash16"][:])
        onescol = dsb.tile([P, 1], F32, tag="onescol")
        nc.vector.memset(onescol[:], 1.0)
        ones1r = dsb.tile([1, P], F32, tag="ones1r")
        nc.vector.memset(ones1r[:], 1.0)
        ones16r = dsb.tile([1, 16], F32, tag="ones16r")
        nc.vector.memset(ones16r[:], 1.0)

        # Per-(tile, expert) totals: [1, NT*E] via ones-column matmul.
        pt = dps.tile([1, NT * E], F32, tag="pt", bufs=1)
        nc.tensor.matmul(
            pt[:], onescol[:], mask[:].rearrange("p i e -> p (i e)"),
            start=True, stop=True,
        )
        tot_sb = dsb.tile([1, NT, E], F32, tag="tot_sb")
        nc.vector.tensor_copy(tot_sb[:].rearrange("p i e -> p (i e)"), pt[:])
        # Reshape to [NT, E] (partition = tile index) via SBUF->SBUF DMA.
        tot16 = dsb.tile([NT, E], F32, tag="tot16")
        nc.sync.dma_start(tot16[:, None, :], tot_sb[0:1, :, :])
        # Exclusive block offsets + expert region base.
        bo_ps = dps.tile([NT, E], F32, tag="bo_ps", bufs=1)
        nc.tensor.matmul(bo_ps[:], tri16s[:], tot16[:], start=True, stop=False)
        nc.tensor.matmul(bo_ps[:], ones16r[:], capoff[:], start=False, stop=True)
        bo_sb = dsb.tile([NT, E], F32, tag="bo_sb")
        nc.vector.tensor_copy(bo_sb[:], bo_ps[:])
        # Matmul rhs base partition must be 0: move rows to partition 0.
        bo_row = dsb.tile([1, NT, E], F32, tag="bo_row")
        nc.sync.dma_start(bo_row[0:1, :, :], bo_sb[:, None, :])

        # Per-token destination slot (or per-partition trash row).
        pos = dsb.tile([P, NT, E], F32, tag="pos")
        for i in range(NT):
            pf = dps.tile([P, E], F32, tag="pf", bufs=2)
            nc.tensor.matmul(pf[:], tri128[:], mask[:, i, :], start=True, stop=False)
            nc.tensor.matmul(pf[:], ones1r[:], bo_row[0:1, i, :], start=False, stop=True)
            nc.vector.tensor_tensor(pos[:, i, :], pf[:], mask[:, i, :], op=ALU.subtract)
        d1 = dsb.tile([P, NT, E], F32, tag="d1")
        nc.vector.tensor_tensor(
            d1[:], pos[:], trashv[:].to_broadcast([P, NT, E]), op=ALU.subtract
        )
        nc.vector.tensor_tensor(d1[:], d1[:], mask[:], op=ALU.mult)
        destf = dsb.tile([P, NT, E], F32, tag="destf")
        nc.vector.tensor_tensor(
            destf[:], d1[:], trashv[:].to_broadcast([P, NT, E]), op=ALU.add
        )
        # Permute to (p, e, i) on DVE (with int16 cast), then rewrap via DRAM:
        # flat f = e*T + i*128 + p -> [16, f/16] x8 groups.
        dest16 = dsb.tile([P, E, NT], mybir.dt.int16, tag="dest16")
        nc.vector.tensor_copy(dest16[:], destf[:].rearrange("p i e -> p e i"))
        nc.sync.dma_start(
            io["dscratch"][:].rearrange("(e i p) -> p e i", p=P, i=NT), dest16[:]
        )
        idxs16 = dsb.tile([P, (T * E) // 16], mybir.dt.int16, tag="idxs16")
        for g in range(8):
            nc.sync.dma_start(
                idxs16[g * 16 : (g + 1) * 16, :],
                io["dscratch"][:].rearrange("(c p) -> p c", p=16),
            )

        # Scatter payload: col0 = t+1, col1 = 1 + gate, rest 0.
        in_tile = dsb.tile([P, P, CP], F32, tag="in_tile")
        nc.vector.memset(in_tile[:], 0.0)
        nc.vector.tensor_copy(
            in_tile[:, :, 0:1].rearrange("p j c -> p (j c)"), idconst[:]
        )
        nc.vector.tensor_scalar(
            in_tile[:, :, 1:2].rearrange("p (e i) c -> p e i c", e=E),
            gates[:].rearrange("p i e -> p e i")[:, :, :, None],
            1.0, None, op0=ALU.add,
        )

        # idxlist init to -1, then scatter-add ids+gates.
        zi = dsb.tile([P, ((NSLOT + P) * CP) // P], F32, tag="zi")
        nc.vector.memset(zi[:], -1.0)
        nc.sync.dma_start(
            io["idxlist"][:].rearrange("(j p) c -> p j c", p=P),
            zi[:].rearrange("p (j c) -> p j c", c=CP),
        )
        for c in range((T * E) // 512):
            nc.gpsimd.dma_scatter_add(
                io["idxlist"][:], in_tile[:, c * 4 : (c + 1) * 4, :],
                idxs16[:, c * 32 : (c + 1) * 32], 512, 512, CP,
            )

        # Read back: per-slot gates (slot-major [128, NSLOT/128]) ...
        nc.sync.dma_start(
            gsl[:], io["idxlist"][: NSLOT, 1:2].rearrange("(j p) c -> p j c", p=P)
        )
        # ... and raw token ids wrapped [16, NSLOT/16], replicated x8.
        rawidx = dsb.tile([P, NSLOT // 16], F32, tag="rawidx")
        for g in range(8):
            nc.sync.dma_start(
                rawidx[g * 16 : (g + 1) * 16, :],
                io["idxlist"][: NSLOT, 0:1].rearrange("(c p) z -> p (c z)", p=16),
            )
        # Gather ids: clamp padding (-1) to token 0 (valid data, gate unused).
        gclamp = dsb.tile([P, NSLOT // 16], F32, tag="gclamp")
        nc.vector.tensor_scalar_max(gclamp[:], rawidx[:], 0.0)
        nc.vector.tensor_copy(gidx16[:], gclamp[:])
        # Scatter ids: padding goes to trash token rows T + p%16.
        neg = dsb.tile([P, NSLOT // 16], F32, tag="neg")
        nc.vector.tensor_scalar(neg[:], rawidx[:], 0.0, None, op0=ALU.is_lt)
        nc.vector.tensor_tensor(
            neg[:], neg[:], trash16[:].to_broadcast([P, NSLOT // 16]), op=ALU.mult
        )
        nc.vector.tensor_tensor(neg[:], neg[:], gclamp[:], op=ALU.add)
        nc.vector.tensor_copy(sidx16[:], neg[:])

    return gidx16, sidx16, gsl


def _experts_sparse(nc, tc, io, pools, gidx16, sidx16, gsl):  # noqa: C901
    """Sparse expert compute: gather top-2 routed tokens per expert into
    capacity-padded slots, run the MLP in bf16, scale by gates, scatter-add
    into the padded output table."""
    with (
        tc.tile_pool(name="esb", bufs=1) as esb,
        tc.tile_pool(name="ew", bufs=1) as ew,
        tc.tile_pool(name="eps", bufs=1, space="PSUM") as eps,
    ):
        onesb = esb.tile([1, P], BF16, tag="onesb")
        nc.vector.memset(onesb[:], 1.0)
        # Zero the padded output table.
        zo = esb.tile([P, (NPAD * CP) // P], F32, tag="zo")
        nc.vector.memset(zo[:], 0.0)
        nc.sync.dma_start(
            io["outpad"][:].rearrange("(j p) c -> p j c", p=P),
            zo[:].rearrange("p (j c) -> p j c", c=CP),
        )

        for e in range(E):
            eb1_sb = ew.tile([P, H1 // P], F32, tag="eb1", bufs=2, name="eb1_sb")
            nc.sync.dma_start(
                eb1_sb[:], io[f"eb1_{e}"][:].rearrange("(m p) -> p m", p=P)
            )
            eb2_sb = ew.tile([P, H2 // P], F32, tag="eb2", bufs=2, name="eb2_sb")
            nc.sync.dma_start(
                eb2_sb[:], io[f"eb2_{e}"][:].rearrange("(m p) -> p m", p=P)
            )
            w3_sb = ew.tile([P, H2 // P, C], BF16, tag="w3", bufs=2, name="w3_sb")
            nc.sync.dma_start(
                w3_sb[:], io[f"eW3_{e}"][:].rearrange("(k p) m -> p k m", p=P)
            )
            b3_sb = ew.tile([1, C], BF16, tag="b3", bufs=2, name="b3_sb")
            nc.sync.dma_start(b3_sb[:], io[f"eb3b_{e}"][:])

            w1_view = io[f"eW1_{e}"][:].rearrange("(k p) m -> p k m", p=P)
            w2_view = io[f"eW2_{e}"][:].rearrange("(k p) m -> p k m", p=P)

            for co in range(0, CAPS[e], CHUNK):
                cs = min(CHUNK, CAPS[e] - co)
                base = OFFS[e] + co
                nsplits = [(0, min(cs, 512))] + (
                    [(512, cs - 512)] if cs > 512 else []
                )
                xg = esb.tile([P, D // P, cs], BF16, tag="xg", bufs=2, name="xg")
                for j in range(cs // P):
                    xstage = esb.tile(
                        [P, D // P, P], BF16, tag="xstage", bufs=3, name="xstage"
                    )
                    nc.gpsimd.dma_gather(
                        xstage[:],
                        io["x_bf"][:],
                        gidx16[:, (base + j * P) // 16 : (base + (j + 1) * P) // 16],
                        P, P, D, transpose=True,
                    )
                    nc.sync.dma_start(xg[:, :, j * P : (j + 1) * P], xstage[:])
                h1g = esb.tile([P, H1 // P, cs], BF16, tag="h1g", bufs=1, name="h1g")
                h2g = esb.tile([P, H2 // P, cs], BF16, tag="h2g", bufs=1, name="h2g")
                # L1
                for mi in range(H1 // 512):
                    w1s = ew.tile([P, D // P, 512], BF16, tag="w1s", bufs=3, name="w1s")
                    nc.sync.dma_start(w1s[:], w1_view[:, :, bass.ts(mi, 512)])
                    for mm in range(4):
                        m = mi * 4 + mm
                        for (n0, nn) in nsplits:
                            ps = eps.tile([P, 512], F32, tag="l1ps", bufs=3, name="l1ps")
                            for k in range(D // P):
                                nc.tensor.matmul(
                                    ps[:, :nn],
                                    w1s[:, k, bass.ts(mm, P)],
                                    xg[:, k, n0 : n0 + nn],
                                    start=(k == 0), stop=(k == D // P - 1),
                                )
                            nc.scalar.activation(
                                h1g[:, m, n0 : n0 + nn], ps[:, :nn],
                                AF.Relu, bias=eb1_sb[:, m : m + 1],
                            )
                # L2
                for mi in range(H2 // 512):
                    w2s = ew.tile([P, H1 // P, 512], BF16, tag="w2s", bufs=2, name="w2s")
                    nc.sync.dma_start(w2s[:], w2_view[:, :, bass.ts(mi, 512)])
                    for mm in range(4):
                        m = mi * 4 + mm
                        for (n0, nn) in nsplits:
                            ps = eps.tile([P, 512], F32, tag="l2ps", bufs=3, name="l2ps")
                            for k in range(H1 // P):
                                nc.tensor.matmul(
                                    ps[:, :nn],
                                    w2s[:, k, bass.ts(mm, P)],
                                    h1g[:, k, n0 : n0 + nn],
                                    start=(k == 0), stop=(k == H1 // P - 1),
                                )
                            nc.scalar.activation(
                                h2g[:, m, n0 : n0 + nn], ps[:, :nn],
                                AF.Relu, bias=eb2_sb[:, m : m + 1],
                            )
                # L3 + gate scale
                sout = ew.tile([P, cs // P, CP], F32, tag="sout", bufs=2, name="sout")
                nc.vector.memset(sout[:], 0.0)
                for j in range(cs // P):
                    ps3 = eps.tile([P, CP], F32, tag="l3ps", bufs=2, name="ps3")
                    for k in range(H2 // P):
                        nc.tensor.matmul(
                            ps3[:, :C],
                            h2g[:, k, bass.ts(j, P)],
                            w3_sb[:, k, :],
                            start=(k == 0), stop=False,
                        )
                    nc.tensor.matmul(
                        ps3[:, :C], onesb[:], b3_sb[:], start=False, stop=True
                    )
                    nc.scalar.activation(
                        sout[:, j, :C], ps3[:, :C], AF.Copy,
                        scale=gsl[:, base // P + j, 0:1],
                    )
                nc.gpsimd.dma_scatter_add(
                    io["outpad"][:], sout[:],
                    sidx16[:, base // 16 : (base + cs) // 16],
                    cs, cs, CP,
                )

        # outpad[:T, :50] -> outbuf[:, :50] via SBUF; probs from acc.
        cp_sb = esb.tile([P, NT, C], F32, tag="cp_sb")
        nc.sync.dma_start(
            cp_sb[:], io["outpad"][:T, :C].rearrange("(i p) c -> p i c", p=P)
        )
        nc.sync.dma_start(
            io["outbuf"][:, :C].rearrange("(i p) c -> p i c", p=P), cp_sb[:]
        )
        acc = pools["acc"]
        nc.sync.dma_start(
            io["outbuf"][:, C : C + E].rearrange("(i p) e -> p i e", p=P),
            acc[:, :, C : C + E],
        )


def build(variant=None):
    variant = variant or VARIANT
    nc = bacc.Bacc(None, target_bir_lowering=False, debug=False)

    io = {}
    io["xT_f32"] = nc.dram_tensor("xT_f32", [D, T], F32, kind="ExternalInput")
    if variant == "dense":
        io["xT_bf16"] = nc.dram_tensor("xT_bf16", [D, T], BF16, kind="ExternalInput")
    else:
        io["x_bf"] = nc.dram_tensor("x_bf", [T, D], BF16, kind="ExternalInput")
        io["tri128"] = nc.dram_tensor("tri128", [P, P], F32, kind="ExternalInput")
        io["tri16s"] = nc.dram_tensor("tri16s", [16, 16], F32, kind="ExternalInput")
        io["idconst"] = nc.dram_tensor("idconst", [P, P], F32, kind="ExternalInput")
        io["capoff"] = nc.dram_tensor("capoff", [1, E], F32, kind="ExternalInput")
        io["trashv"] = nc.dram_tensor("trashv", [P, 1, 1], F32, kind="ExternalInput")
        io["trash16"] = nc.dram_tensor("trash16", [P, 1], F32, kind="ExternalInput")
        io["idxlist"] = nc.dram_tensor("idxlist", [NSLOT + P, CP], F32)
        io["dscratch"] = nc.dram_tensor("dscratch", [T * E], mybir.dt.int16)
        io["outpad"] = nc.dram_tensor("outpad", [NPAD, CP], F32)
    io["rW1"] = nc.dram_tensor("rW1", [D, RH], F32, kind="ExternalInput")
    io["rb1"] = nc.dram_tensor("rb1", [RH], F32, kind="ExternalInput")
    io["rW2"] = nc.dram_tensor("rW2", [RH, RH // 2], F32, kind="ExternalInput")
    io["rb2"] = nc.dram_tensor("rb2", [RH // 2], F32, kind="ExternalInput")
    io["rW3"] = nc.dram_tensor("rW3", [RH // 2, E], F32, kind="ExternalInput")
    io["rb3"] = nc.dram_tensor("rb3", [E], F32, kind="ExternalInput")
    for e in range(E):
        io[f"eW1_{e}"] = nc.dram_tensor(f"eW1_{e}", [D, H1], BF16, kind="ExternalInput")
        io[f"eb1_{e}"] = nc.dram_tensor(f"eb1_{e}", [H1], F32, kind="ExternalInput")
        io[f"eW2_{e}"] = nc.dram_tensor(f"eW2_{e}", [H1, H2], BF16, kind="ExternalInput")
        io[f"eb2_{e}"] = nc.dram_tensor(f"eb2_{e}", [H2], F32, kind="ExternalInput")
        io[f"eW3_{e}"] = nc.dram_tensor(f"eW3_{e}", [H2, C], BF16, kind="ExternalInput")
        io[f"eb3b_{e}"] = nc.dram_tensor(f"eb3b_{e}", [1, C], BF16, kind="ExternalInput")
    io["outbuf"] = nc.dram_tensor("outbuf", [T, CP], F32, kind="ExternalOutput")

    with tile.TileContext(nc) as tc:
        with tc.tile_pool(name="gates_pool", bufs=1) as gpool:
            acc = gpool.tile([P, NT, CP], F32, tag="acc", bufs=1)
            nc.vector.memset(acc[:], 0.0)
            pools = {"gates": gpool, "acc": acc}
            if variant == "dense":
                gates = _router(nc, tc, io, pools)
                _experts(nc, tc, io, pools, gates)
            else:
                from concourse import library_config
                nc.gpsimd.load_library(library_config.mlp)
                gates, mask = _router(nc, tc, io, pools, want_mask=True)
                gidx16, sidx16, gsl = _dispatch(nc, tc, io, pools, gates, mask)
                _experts_sparse(nc, tc, io, pools, gidx16, sidx16, gsl)

    nc.compile()
    return nc


def _get_built(caps=None):
    key = (VARIANT, tuple(caps) if caps else None)
    if key not in _built:
        if caps:
            set_caps(caps)
        _built[key] = build()
    return _built[key]


def make_in_maps(inputs, variant=None):
    """Shard FULL inputs into per-core in_maps."""
    variant = variant or VARIANT
    x = np.asarray(inputs["x"], np.float32)
    eW1 = np.asarray(inputs["eW1"], np.float32)
    eW2 = np.asarray(inputs["eW2"], np.float32)
    eW3 = np.asarray(inputs["eW3"], np.float32)
    eb1 = np.asarray(inputs["eb1"], np.float32)
    eb2 = np.asarray(inputs["eb2"], np.float32)
    eb3 = np.asarray(inputs["eb3"], np.float32)
    shared = {}
    for k in ("rW1", "rb1", "rW2", "rb2", "rW3", "rb3"):
        shared[k] = np.ascontiguousarray(np.asarray(inputs[k], np.float32))
    for e in range(E):
        shared[f"eW1_{e}"] = np.ascontiguousarray(eW1[e].astype(ml_dtypes.bfloat16))
        shared[f"eW2_{e}"] = np.ascontiguousarray(eW2[e].astype(ml_dtypes.bfloat16))
        shared[f"eW3_{e}"] = np.ascontiguousarray(eW3[e].astype(ml_dtypes.bfloat16))
        shared[f"eb1_{e}"] = np.ascontiguousarray(eb1[e])
        shared[f"eb2_{e}"] = np.ascontiguousarray(eb2[e])
        shared[f"eb3b_{e}"] = np.ascontiguousarray(
            eb3[e].astype(ml_dtypes.bfloat16)[None, :]
        )
    if variant == "sparse":
        p = np.arange(P, dtype=np.float32)
        shared["tri128"] = (
            (np.arange(P)[:, None] <= np.arange(P)[None, :]).astype(np.float32)
        )
        shared["tri16s"] = (
            (np.arange(16)[:, None] < np.arange(16)[None, :]).astype(np.float32)
        )
        shared["idconst"] = np.ascontiguousarray(
            1.0 + (np.arange(P)[None, :] % 16) * 128 + p[:, None]
        ).astype(np.float32)
        shared["capoff"] = np.asarray(OFFS, np.float32)[None, :]
        shared["trashv"] = (NSLOT + p).astype(np.float32)[:, None, None]
        shared["trash16"] = (T + np.arange(P, dtype=np.float32) % 16)[:, None]

    in_maps = []
    for c in range(NCORES):
        xs = x[c * T : (c + 1) * T]
        xT = np.ascontiguousarray(xs.T)
        m = dict(shared)
        m["xT_f32"] = xT
        if variant == "dense":
            m["xT_bf16"] = xT.astype(ml_dtypes.bfloat16)
        else:
            m["x_bf"] = xs.astype(ml_dtypes.bfloat16)
        in_maps.append(m)
    return in_maps


def kernel(**inputs):
    assert int(inputs.get("top_k", 2)) == 2
    caps = compute_caps(inputs) if VARIANT == "sparse" else None
    nc = _get_built(caps)
    in_maps = make_in_maps(inputs)
    res = run_bass_kernel_spmd(nc, in_maps, core_ids=list(range(NCORES)))
    buf = np.concatenate([res.results[c]["outbuf"] for c in range(NCORES)], axis=0)
    return np.ascontiguousarray(buf[:, :C]), np.ascontiguousarray(buf[:, C : C + E])


# revision 43
# speedup vs baseline: 2.9740x; 2.9740x over previous
"""MoE model kernel for Trainium2 (8 NeuronCores, data-parallel over batch).

Reference computation (per token):
  router: 3-layer MLP (fp32) -> softmax -> top-2 gates (vals/2 scattered dense)
  experts: 8x (D->H1 relu, H1->H2 relu, H2->C) combined with gates
Outputs: (out [B, C] f32, probs [B, E] f32)

Sharding: batch B=16384 split across 8 cores (2048 tokens each); router and
all experts replicated on every core. Router runs in fp32 on the PE (top-2
selection needs fp32 accuracy: min prob gap between ranks 2/3 is ~1.5e-6).
Expert matmuls run in bf16 with fp32 PSUM accumulation (max err ~0.3% of
output scale). The dense gated combine matches the reference's math exactly
(gates are 0 for non-selected experts).
"""

import numpy as np
import ml_dtypes

import concourse.bacc as bacc
import concourse.bass as bass
import concourse.mybir as mybir
import concourse.tile as tile
from concourse.bass_utils import run_bass_kernel_spmd

F32 = mybir.dt.float32
BF16 = mybir.dt.bfloat16
AF = mybir.ActivationFunctionType
ALU = mybir.AluOpType
AX = mybir.AxisListType

B, D, E, C = 16384, 1024, 8, 50
RH = 512
H1, H2 = 2048, 1024
NCORES = 8
T = B // NCORES  # tokens per core
P = 128
NT = T // P      # 16 token tiles per core
CP = 64          # padded C for psum/acc tiles

# Sparse dispatch: per-expert slot capacities (multiples of 128), computed
# per input-set on the host (the routing distribution depends on the actual
# router weights/data, and the jax PRNG stream is backend-dependent). Tokens
# beyond capacity would be silently dropped, so capacities carry a >=25%
# + 128 margin over the max per-core count; compiled kernels are cached per
# capacity profile.
CAPS = (768, 1536, 256, 256, 768, 1152, 1152, 512)   # default profile
OFFS = tuple(int(np.sum(CAPS[:e])) for e in range(E))
NSLOT = int(np.sum(CAPS))
NPAD = T + P                        # outpad rows (last 128 = trash tokens)
CHUNK = 768                         # max slots per compute chunk

VARIANT = "sparse"

_built = {}


def set_caps(caps):
    global CAPS, OFFS, NSLOT
    CAPS = tuple(int(c) for c in caps)
    assert all(c % P == 0 and c > 0 for c in CAPS)
    OFFS = tuple(int(np.sum(CAPS[:e])) for e in range(E))
    NSLOT = int(np.sum(CAPS))


def compute_caps(inputs):
    """Host-side router pass -> per-expert capacities (max core count +25%+128,
    rounded up to 128)."""
    x = np.asarray(inputs["x"], np.float32)
    h = np.maximum(x @ np.asarray(inputs["rW1"], np.float32)
                   + np.asarray(inputs["rb1"], np.float32), 0)
    h = np.maximum(h @ np.asarray(inputs["rW2"], np.float32)
                   + np.asarray(inputs["rb2"], np.float32), 0)
    s = h @ np.asarray(inputs["rW3"], np.float32) + np.asarray(inputs["rb3"], np.float32)
    # top-2 selection only needs score order; softmax is monotone per row.
    thresh = np.sort(s, axis=1)[:, -2:-1]
    sel = s >= thresh
    counts = np.zeros((NCORES, E), np.int64)
    for c in range(NCORES):
        counts[c] = sel[c * T : (c + 1) * T].sum(0)
    worst = counts.max(0)
    caps = np.ceil((worst + 64) / P).astype(np.int64) * P
    caps = np.minimum(caps, T)
    return tuple(int(v) for v in caps)


def _router(nc, tc, io, pools, want_mask=False):
    """Router MLP in fp32 + softmax + top-2 gates.

    Writes probs into acc[:, :, 50:58] (packed output). Returns the gates
    tile [128, NT, E] f32 (token-major) in a long-lived pool (and the 0/1
    top-2 mask when want_mask).
    """
    gpool = pools["gates"]
    acc = pools["acc"]
    gates = gpool.tile([P, NT, E], F32, tag="gates", bufs=1)
    probs_sb = gpool.tile([P, NT, E], F32, tag="probs", bufs=1)
    mask = None
    if want_mask:
        mask = gpool.tile([P, NT, E], F32, tag="mask", bufs=1, name="mask")

    with (
        tc.tile_pool(name="rsb", bufs=1) as rsb,
        tc.tile_pool(name="rstream", bufs=3) as rstream,
        tc.tile_pool(name="rps", bufs=1, space="PSUM") as rps,
    ):
        # Router weights resident in SBUF (fp32, ~2.6MB total)
        rW1_sb = rsb.tile([P, D // P, RH], F32, tag="rW1")
        nc.sync.dma_start(rW1_sb[:], io["rW1"][:].rearrange("(k p) m -> p k m", p=P))
        rW2_sb = rsb.tile([P, RH // P, RH // 2], F32, tag="rW2")
        nc.sync.dma_start(rW2_sb[:], io["rW2"][:].rearrange("(k p) m -> p k m", p=P))
        rW3_sb = rsb.tile([P, (RH // 2) // P, E], F32, tag="rW3")
        nc.sync.dma_start(rW3_sb[:], io["rW3"][:].rearrange("(k p) m -> p k m", p=P))
        rb1_sb = rsb.tile([P, RH // P], F32, tag="rb1")
        nc.sync.dma_start(rb1_sb[:], io["rb1"][:].rearrange("(m p) -> p m", p=P))
        rb2_sb = rsb.tile([P, (RH // 2) // P], F32, tag="rb2")
        nc.sync.dma_start(rb2_sb[:], io["rb2"][:].rearrange("(m p) -> p m", p=P))
        rb3_sb = rsb.tile([1, E], F32, tag="rb3")
        nc.sync.dma_start(rb3_sb[:], io["rb3"][None, :])
        ones1 = rsb.tile([1, P], F32, tag="ones1")
        nc.vector.memset(ones1[:], 1.0)

        h1r = rsb.tile([P, RH // P, T], F32, tag="h1r")   # 4MB
        h2r = rsb.tile([P, (RH // 2) // P, T], F32, tag="h2r")  # 2MB

        NTOK = 512  # token tile for router layers
        xT_view = io["xT_f32"][:].rearrange("(k p) t -> p k t", p=P)

        # L1: h1r = relu(rW1.T @ xT + rb1)
        for n in range(T // NTOK):
            nsl = bass.ts(n, NTOK)
            xks = []
            for k in range(D // P):
                xk = rstream.tile([P, NTOK], F32, tag=f"xk{k % 2}", bufs=2)
                nc.sync.dma_start(xk[:], xT_view[:, k, nsl])
                xks.append(xk)
            psl1 = [
                rps.tile([P, NTOK], F32, tag=f"psl1_{m}", bufs=1, name=f"psl1_{m}")
                for m in range(RH // P)
            ]
            for k in range(D // P):
                for m in range(RH // P):
                    nc.tensor.matmul(
                        psl1[m][:],
                        rW1_sb[:, k, bass.ts(m, P)],
                        xks[k][:],
                        start=(k == 0),
                        stop=(k == D // P - 1),
                    )
            for m in range(RH // P):
                nc.scalar.activation(
                    h1r[:, m, nsl], psl1[m][:], AF.Relu, bias=rb1_sb[:, m : m + 1]
                )

        # L2: h2r = relu(rW2.T @ h1r + rb2)
        for n in range(T // NTOK):
            nsl = bass.ts(n, NTOK)
            psl2 = [
                rps.tile([P, NTOK], F32, tag=f"psl2_{m}", bufs=1, name=f"psl2_{m}")
                for m in range((RH // 2) // P)
            ]
            for k in range(RH // P):
                for m in range((RH // 2) // P):
                    nc.tensor.matmul(
                        psl2[m][:],
                        rW2_sb[:, k, bass.ts(m, P)],
                        h1r[:, k, nsl],
                        start=(k == 0),
                        stop=(k == RH // P - 1),
                    )
            for m in range((RH // 2) // P):
                nc.scalar.activation(
                    h2r[:, m, nsl], psl2[m][:], AF.Relu, bias=rb2_sb[:, m : m + 1]
                )

        # L3 (token-major): scores[t, e] = h2r.T @ rW3 + rb3
        scores = gpool.tile([P, NT, E], F32, tag="scores", bufs=1)
        for i in range(NT):
            ps3 = rps.tile([P, E], F32, tag="ps3", bufs=2)
            for k in range((RH // 2) // P):
                nc.tensor.matmul(
                    ps3[:],
                    h2r[:, k, bass.ts(i, P)],
                    rW3_sb[:, k, :],
                    start=(k == 0),
                    stop=False,
                )
            nc.tensor.matmul(ps3[:], ones1[:], rb3_sb[:], start=False, stop=True)
            nc.vector.tensor_copy(scores[:, i, :], ps3[:])

        # Softmax over E (per 8-wide segment)
        mx = rsb.tile([P, NT, 1], F32, tag="mx")
        nc.vector.tensor_reduce(mx[:], scores[:], axis=AX.X, op=ALU.max)
        xs = rsb.tile([P, NT, E], F32, tag="xs")
        nc.vector.tensor_tensor(
            xs[:], scores[:], mx[:].to_broadcast([P, NT, E]), op=ALU.subtract
        )
        ex = rsb.tile([P, NT, E], F32, tag="ex")
        nc.scalar.activation(ex[:], xs[:], AF.Exp)
        sm = rsb.tile([P, NT, 1], F32, tag="sm")
        nc.vector.tensor_reduce(sm[:], ex[:], axis=AX.X, op=ALU.add)
        rs = rsb.tile([P, NT, 1], F32, tag="rs")
        nc.vector.reciprocal(rs[:], sm[:])
        nc.vector.tensor_tensor(
            probs_sb[:], ex[:], rs[:].to_broadcast([P, NT, E]), op=ALU.mult
        )
        nc.vector.tensor_copy(acc[:, :, C : C + E], probs_sb[:])

        # Top-2 gates: gates = probs * (probs >= 2nd_max) / 2
        m1 = rsb.tile([P, NT, 1], F32, tag="m1")
        nc.vector.tensor_reduce(m1[:], probs_sb[:], axis=AX.X, op=ALU.max)
        lt = rsb.tile([P, NT, E], F32, tag="lt")
        nc.vector.tensor_tensor(
            lt[:], probs_sb[:], m1[:].to_broadcast([P, NT, E]), op=ALU.is_lt
        )
        pz = rsb.tile([P, NT, E], F32, tag="pz")
        nc.vector.tensor_tensor(pz[:], probs_sb[:], lt[:], op=ALU.mult)
        m2 = rsb.tile([P, NT, 1], F32, tag="m2")
        nc.vector.tensor_reduce(m2[:], pz[:], axis=AX.X, op=ALU.max)
        ge = mask if want_mask else rsb.tile([P, NT, E], F32, tag="ge")
        nc.vector.tensor_tensor(
            ge[:], probs_sb[:], m2[:].to_broadcast([P, NT, E]), op=ALU.is_ge
        )
        ph = rsb.tile([P, NT, E], F32, tag="ph")
        nc.vector.tensor_scalar_mul(ph[:], probs_sb[:], 0.5)
        nc.vector.tensor_tensor(gates[:], ph[:], ge[:], op=ALU.mult)

    if want_mask:
        return gates, mask
    return gates


def _experts(nc, tc, io, pools, gates):
    """Dense expert compute in bf16 with gated fp32 combine."""
    acc = pools["acc"]

    TH = 1024       # tokens per half
    NTOK = 512      # matmul free dim
    NH = T // TH    # 2 halves

    with (
        tc.tile_pool(name="esb", bufs=1) as esb,
        tc.tile_pool(name="ew", bufs=1) as ew,
        tc.tile_pool(name="eps", bufs=1, space="PSUM") as eps,
    ):
        xTb_sb = esb.tile([P, D // P, T], BF16, tag="xTb")  # 4MB resident
        nc.sync.dma_start(
            xTb_sb[:], io["xT_bf16"][:].rearrange("(k p) t -> p k t", p=P)
        )
        onesb = esb.tile([1, P], BF16, tag="onesb")
        nc.vector.memset(onesb[:], 1.0)
        h1b = esb.tile([P, H1 // P, TH], BF16, tag="h1b")  # 4MB
        h2b = esb.tile([P, H2 // P, TH], BF16, tag="h2b")  # 2MB

        for e in range(E):
            eb1_sb = ew.tile([P, H1 // P], F32, tag="eb1", bufs=2)
            nc.sync.dma_start(
                eb1_sb[:], io[f"eb1_{e}"][:].rearrange("(m p) -> p m", p=P)
            )
            eb2_sb = ew.tile([P, H2 // P], F32, tag="eb2", bufs=2)
            nc.sync.dma_start(
                eb2_sb[:], io[f"eb2_{e}"][:].rearrange("(m p) -> p m", p=P)
            )
            w3_sb = ew.tile([P, H2 // P, C], BF16, tag="w3", bufs=2)
            nc.sync.dma_start(
                w3_sb[:], io[f"eW3_{e}"][:].rearrange("(k p) m -> p k m", p=P)
            )
            b3_sb = ew.tile([1, C], BF16, tag="b3", bufs=2)
            nc.sync.dma_start(b3_sb[:], io[f"eb3b_{e}"][:])

            w1_view = io[f"eW1_{e}"][:].rearrange("(k p) m -> p k m", p=P)
            w2_view = io[f"eW2_{e}"][:].rearrange("(k p) m -> p k m", p=P)

            for h in range(NH):
                hsl = slice(h * TH, (h + 1) * TH)
                # L1: h1b = relu(W1.T @ x + b1), M=H1 in 4 strips of 512
                for mi in range(H1 // NTOK):
                    w1s = ew.tile([P, D // P, NTOK], BF16, tag="w1s", bufs=3)
                    nc.sync.dma_start(w1s[:], w1_view[:, :, bass.ts(mi, NTOK)])
                    for mm in range(NTOK // P):
                        m = mi * (NTOK // P) + mm
                        for n in range(TH // NTOK):
                            nsl = slice(h * TH + n * NTOK, h * TH + (n + 1) * NTOK)
                            ps = eps.tile([P, NTOK], F32, tag="l1ps", bufs=3)
                            for k in range(D // P):
                                nc.tensor.matmul(
                                    ps[:],
                                    w1s[:, k, bass.ts(mm, P)],
                                    xTb_sb[:, k, nsl],
                                    start=(k == 0),
                                    stop=(k == D // P - 1),
                                )
                            nc.scalar.activation(
                                h1b[:, m, bass.ts(n, NTOK)],
                                ps[:],
                                AF.Relu,
                                bias=eb1_sb[:, m : m + 1],
                            )
                # L2: h2b = relu(W2.T @ h1b + b2), M=H2 in 2 strips of 512
                for mi in range(H2 // NTOK):
                    w2s = ew.tile([P, H1 // P, NTOK], BF16, tag="w2s", bufs=2)
                    nc.sync.dma_start(w2s[:], w2_view[:, :, bass.ts(mi, NTOK)])
                    for mm in range(NTOK // P):
                        m = mi * (NTOK // P) + mm
                        for n in range(TH // NTOK):
                            ps = eps.tile([P, NTOK], F32, tag="l2ps", bufs=3)
                            for k in range(H1 // P):
                                nc.tensor.matmul(
                                    ps[:],
                                    w2s[:, k, bass.ts(mm, P)],
                                    h1b[:, k, bass.ts(n, NTOK)],
                                    start=(k == 0),
                                    stop=(k == H1 // P - 1),
                                )
                            nc.scalar.activation(
                                h2b[:, m, bass.ts(n, NTOK)],
                                ps[:],
                                AF.Relu,
                                bias=eb2_sb[:, m : m + 1],
                            )
                # L3 token-major + gated combine
                for i in range(TH // P):
                    it = h * (TH // P) + i
                    ps3 = eps.tile([P, CP], F32, tag="l3ps", bufs=2)
                    for k in range(H2 // P):
                        nc.tensor.matmul(
                            ps3[:, :C],
                            h2b[:, k, bass.ts(i, P)],
                            w3_sb[:, k, :],
                            start=(k == 0),
                            stop=False,
                        )
                    nc.tensor.matmul(
                        ps3[:, :C], onesb[:], b3_sb[:], start=False, stop=True
                    )
                    tmp = ew.tile([P, CP], F32, tag="tmp", bufs=4)
                    nc.scalar.activation(
                        tmp[:, :C], ps3[:, :C], AF.Copy,
                        scale=gates[:, it, e : e + 1],
                    )
                    nc.vector.tensor_add(
                        acc[:, it, :C], acc[:, it, :C], tmp[:, :C]
                    )

        nc.sync.dma_start(io["outbuf"][:].rearrange("(i p) c -> p i c", p=P), acc[:])


def _dispatch(nc, tc, io, pools, gates, mask):
    """Build per-expert token slot lists from the top-2 mask.

    Slot position of token t in expert e = OFFS[e] + (# tokens < t routed to
    e), computed with triangular-matrix prefix-sum matmuls. Token ids and
    gate values are scatter-added into the idxlist DRAM table (init -1, add
    t+1 / 1+gate), then read back as gather/scatter index lists.

    Returns (gidx16 [128, NSLOT/16] int16 clamped >=0 for gathers,
             sidx16 [128, NSLOT/16] int16 with trash-token padding for the
             output scatter, gsl [128, NSLOT/128, 1] f32 per-slot gates).
    """
    gpool = pools["gates"]
    gidx16 = gpool.tile([P, NSLOT // 16], mybir.dt.int16, tag="gidx16", bufs=1)
    sidx16 = gpool.tile([P, NSLOT // 16], mybir.dt.int16, tag="sidx16", bufs=1)
    gsl = gpool.tile([P, NSLOT // P, 1], F32, tag="gsl", bufs=1)

    with (
        tc.tile_pool(name="dsb", bufs=1) as dsb,
        tc.tile_pool(name="dps", bufs=1, space="PSUM") as dps,
    ):
        tri128 = dsb.tile([P, P], F32, tag="tri128")
        nc.sync.dma_start(tri128[:], io["tri128"][:])
        tri16s = dsb.tile([16, 16], F32, tag="tri16s")
        nc.sync.dma_start(tri16s[:], io["tri16s"][:])
        idconst = dsb.tile([P, P], F32, tag="idconst")
        nc.sync.dma_start(idconst[:], io["idconst"][:])
        capoff = dsb.tile([1, E], F32, tag="capoff")
        nc.sync.dma_start(capoff[:], io["capoff"][:])
        trashv = dsb.tile([P, 1, 1], F32, tag="trashv")
        nc.sync.dma_start(trashv[:], io["trashv"][:])
        trash16 = dsb.tile([P, 1], F32, tag="trash16")
        nc.sync.dma_start(trash16[:], io["trash16"][:])
        onescol = dsb.tile([P, 1], F32, tag="onescol")
        nc.vector.memset(onescol[:], 1.0)
        ones1r = dsb.tile([1, P], F32, tag="ones1r")
        nc.vector.memset(ones1r[:], 1.0)
        ones16r = dsb.tile([1, 16], F32, tag="ones16r")
        nc.vector.memset(ones16r[:], 1.0)

        # Per-(tile, expert) totals: [1, NT*E] via ones-column matmul.
        pt = dps.tile([1, NT * E], F32, tag="pt", bufs=1)
        nc.tensor.matmul(
            pt[:], onescol[:], mask[:].rearrange("p i e -> p (i e)"),
            start=True, stop=True,
        )
        tot_sb = dsb.tile([1, NT, E], F32, tag="tot_sb")
        nc.vector.tensor_copy(tot_sb[:].rearrange("p i e -> p (i e)"), pt[:])
        # Reshape to [NT, E] (partition = tile index) via SBUF->SBUF DMA.
        tot16 = dsb.tile([NT, E], F32, tag="tot16")
        nc.sync.dma_start(tot16[:, None, :], tot_sb[0:1, :, :])
        # Exclusive block offsets + expert region base.
        bo_ps = dps.tile([NT, E], F32, tag="bo_ps", bufs=1)
        nc.tensor.matmul(bo_ps[:], tri16s[:], tot16[:], start=True, stop=False)
        nc.tensor.matmul(bo_ps[:], ones16r[:], capoff[:], start=False, stop=True)
        bo_sb = dsb.tile([NT, E], F32, tag="bo_sb")
        nc.vector.tensor_copy(bo_sb[:], bo_ps[:])
        # Matmul rhs base partition must be 0: move rows to partition 0.
        bo_row = dsb.tile([1, NT, E], F32, tag="bo_row")
        nc.sync.dma_start(bo_row[0:1, :, :], bo_sb[:, None, :])

        # Per-token destination slot (or per-partition trash row).
        pos = dsb.tile([P, NT, E], F32, tag="pos")
        for i in range(NT):
            pf = dps.tile([P, E], F32, tag="pf", bufs=2)
            nc.tensor.matmul(pf[:], tri128[:], mask[:, i, :], start=True, stop=False)
            nc.tensor.matmul(pf[:], ones1r[:], bo_row[0:1, i, :], start=False, stop=True)
            nc.vector.tensor_tensor(pos[:, i, :], pf[:], mask[:, i, :], op=ALU.subtract)
        d1 = dsb.tile([P, NT, E], F32, tag="d1")
        nc.vector.tensor_tensor(
            d1[:], pos[:], trashv[:].to_broadcast([P, NT, E]), op=ALU.subtract
        )
        nc.vector.tensor_tensor(d1[:], d1[:], mask[:], op=ALU.mult)
        destf = dsb.tile([P, NT, E], F32, tag="destf")
        nc.vector.tensor_tensor(
            destf[:], d1[:], trashv[:].to_broadcast([P, NT, E]), op=ALU.add
        )
        # Permute to (p, e, i) on DVE (with int16 cast), then rewrap via DRAM:
        # flat f = e*T + i*128 + p -> [16, f/16] x8 groups.
        dest16 = dsb.tile([P, E, NT], mybir.dt.int16, tag="dest16")
        nc.vector.tensor_copy(dest16[:], destf[:].rearrange("p i e -> p e i"))
        nc.sync.dma_start(
            io["dscratch"][:].rearrange("(e i p) -> p e i", p=P, i=NT), dest16[:]
        )
        idxs16 = dsb.tile([P, (T * E) // 16], mybir.dt.int16, tag="idxs16")
        for g in range(8):
            nc.sync.dma_start(
                idxs16[g * 16 : (g + 1) * 16, :],
                io["dscratch"][:].rearrange("(c p) -> p c", p=16),
            )

        # Scatter payload: col0 = t+1, col1 = 1 + gate (row stride stays CP).
        in_tile = dsb.tile([P, P, 2], F32, tag="in_tile")
        nc.vector.tensor_copy(
            in_tile[:, :, 0:1].rearrange("p j c -> p (j c)"), idconst[:]
        )
        nc.vector.tensor_scalar(
            in_tile[:, :, 1:2].rearrange("p (e i) c -> p e i c", e=E),
            gates[:].rearrange("p i e -> p e i")[:, :, :, None],
            1.0, None, op0=ALU.add,
        )

        # idxlist cols 0:2 init to -1, then scatter-add ids+gates.
        zi = dsb.tile([P, (NSLOT + P) // P, 2], F32, tag="zi")
        nc.vector.memset(zi[:], -1.0)
        nc.sync.dma_start(
            io["idxlist"][:, 0:2].rearrange("(j p) c -> p j c", p=P), zi[:]
        )
        CH = 896  # m2s = 113 <= 128-entry SWDGE ring
        for c0 in range(0, T * E, CH):
            cn = min(CH, T * E - c0)
            nc.gpsimd.dma_scatter_add(
                io["idxlist"][:, 0:2], in_tile[:, c0 // P : (c0 + cn) // P, :],
                idxs16[:, c0 // 16 : (c0 + cn) // 16], cn, cn, 2, elem_step=CP,
            )

        # Read back: per-slot gates (slot-major [128, NSLOT/128]) ...
        nc.sync.dma_start(
            gsl[:], io["idxlist"][: NSLOT, 1:2].rearrange("(j p) c -> p j c", p=P)
        )
        # ... and raw token ids wrapped [16, NSLOT/16], replicated x8.
        rawidx = dsb.tile([P, NSLOT // 16], F32, tag="rawidx")
        for g in range(8):
            nc.sync.dma_start(
                rawidx[g * 16 : (g + 1) * 16, :],
                io["idxlist"][: NSLOT, 0:1].rearrange("(c p) z -> p (c z)", p=16),
            )
        # Gather ids: clamp padding (-1) to token 0 (valid data, gate unused).
        gclamp = dsb.tile([P, NSLOT // 16], F32, tag="gclamp")
        nc.vector.tensor_scalar_max(gclamp[:], rawidx[:], 0.0)
        nc.vector.tensor_copy(gidx16[:], gclamp[:])
        # Scatter ids: padding goes to trash token rows T + p%16.
        neg = dsb.tile([P, NSLOT // 16], F32, tag="neg")
        nc.vector.tensor_scalar(neg[:], rawidx[:], 0.0, None, op0=ALU.is_lt)
        nc.vector.tensor_tensor(
            neg[:], neg[:], trash16[:].to_broadcast([P, NSLOT // 16]), op=ALU.mult
        )
        nc.vector.tensor_tensor(neg[:], neg[:], gclamp[:], op=ALU.add)
        nc.vector.tensor_copy(sidx16[:], neg[:])

    return gidx16, sidx16, gsl


def _experts_sparse(nc, tc, io, pools, gidx16, sidx16, gsl):  # noqa: C901
    """Sparse expert compute: gather top-2 routed tokens per expert into
    capacity-padded slots, run the MLP in bf16, scale by gates, scatter-add
    into the padded output table."""
    with (
        tc.tile_pool(name="esb", bufs=1) as esb,
        tc.tile_pool(name="ew", bufs=1) as ew,
        tc.tile_pool(name="eps", bufs=1, space="PSUM") as eps,
    ):
        onesb = esb.tile([1, P], BF16, tag="onesb")
        nc.vector.memset(onesb[:], 1.0)
        # Zero the padded output table.
        zo = esb.tile([P, (NPAD * CP) // P], F32, tag="zo")
        nc.vector.memset(zo[:], 0.0)
        nc.sync.dma_start(
            io["outpad"][:].rearrange("(j p) c -> p j c", p=P),
            zo[:].rearrange("p (j c) -> p j c", c=CP),
        )

        for e in range(E):
            eb1_sb = ew.tile([P, H1 // P], F32, tag="eb1", bufs=2, name="eb1_sb")
            nc.sync.dma_start(
                eb1_sb[:], io[f"eb1_{e}"][:].rearrange("(m p) -> p m", p=P)
            )
            eb2_sb = ew.tile([P, H2 // P], F32, tag="eb2", bufs=2, name="eb2_sb")
            nc.sync.dma_start(
                eb2_sb[:], io[f"eb2_{e}"][:].rearrange("(m p) -> p m", p=P)
            )
            w3_sb = ew.tile([P, H2 // P, C], BF16, tag="w3", bufs=2, name="w3_sb")
            nc.sync.dma_start(
                w3_sb[:], io[f"eW3_{e}"][:].rearrange("(k p) m -> p k m", p=P)
            )
            b3_sb = ew.tile([1, C], BF16, tag="b3", bufs=2, name="b3_sb")
            nc.sync.dma_start(b3_sb[:], io[f"eb3b_{e}"][:])

            w1_view = io[f"eW1_{e}"][:].rearrange("(k p) m -> p k m", p=P)
            w2_view = io[f"eW2_{e}"][:].rearrange("(k p) m -> p k m", p=P)

            for co in range(0, CAPS[e], CHUNK):
                cs = min(CHUNK, CAPS[e] - co)
                base = OFFS[e] + co
                nsplits = [(0, min(cs, 512))] + (
                    [(512, cs - 512)] if cs > 512 else []
                )
                xg = esb.tile([P, D // P, cs], BF16, tag="xg", bufs=2, name="xg")
                for j in range(cs // P):
                    xstage = esb.tile(
                        [P, D // P, P], BF16, tag="xstage", bufs=3, name="xstage"
                    )
                    nc.gpsimd.dma_gather(
                        xstage[:],
                        io["x_bf"][:],
                        gidx16[:, (base + j * P) // 16 : (base + (j + 1) * P) // 16],
                        P, P, D, transpose=True,
                    )
                    nc.sync.dma_start(xg[:, :, j * P : (j + 1) * P], xstage[:])
                h1g = esb.tile([P, H1 // P, cs], BF16, tag="h1g", bufs=1, name="h1g")
                h2g = esb.tile([P, H2 // P, cs], BF16, tag="h2g", bufs=1, name="h2g")
                # L1
                for mi in range(H1 // 512):
                    w1s = ew.tile([P, D // P, 512], BF16, tag="w1s", bufs=3, name="w1s")
                    nc.sync.dma_start(w1s[:], w1_view[:, :, bass.ts(mi, 512)])
                    for mm in range(4):
                        m = mi * 4 + mm
                        for (n0, nn) in nsplits:
                            ps = eps.tile([P, 512], F32, tag="l1ps", bufs=3, name="l1ps")
                            for k in range(D // P):
                                nc.tensor.matmul(
                                    ps[:, :nn],
                                    w1s[:, k, bass.ts(mm, P)],
                                    xg[:, k, n0 : n0 + nn],
                                    start=(k == 0), stop=(k == D // P - 1),
                                )
                            nc.scalar.activation(
                                h1g[:, m, n0 : n0 + nn], ps[:, :nn],
                                AF.Relu, bias=eb1_sb[:, m : m + 1],
                            )
                # L2
                for mi in range(H2 // 512):
                    w2s = ew.tile([P, H1 // P, 512], BF16, tag="w2s", bufs=2, name="w2s")
                    nc.sync.dma_start(w2s[:], w2_view[:, :, bass.ts(mi, 512)])
                    for mm in range(4):
                        m = mi * 4 + mm
                        for (n0, nn) in nsplits:
                            ps = eps.tile([P, 512], F32, tag="l2ps", bufs=3, name="l2ps")
                            for k in range(H1 // P):
                                nc.tensor.matmul(
                                    ps[:, :nn],
                                    w2s[:, k, bass.ts(mm, P)],
                                    h1g[:, k, n0 : n0 + nn],
                                    start=(k == 0), stop=(k == H1 // P - 1),
                                )
                            nc.scalar.activation(
                                h2g[:, m, n0 : n0 + nn], ps[:, :nn],
                                AF.Relu, bias=eb2_sb[:, m : m + 1],
                            )
                # L3 + gate scale
                sout = ew.tile([P, cs // P, C], F32, tag="sout", bufs=2, name="sout")
                for j in range(cs // P):
                    ps3 = eps.tile([P, CP], F32, tag="l3ps", bufs=2, name="ps3")
                    for k in range(H2 // P):
                        nc.tensor.matmul(
                            ps3[:, :C],
                            h2g[:, k, bass.ts(j, P)],
                            w3_sb[:, k, :],
                            start=(k == 0), stop=False,
                        )
                    nc.tensor.matmul(
                        ps3[:, :C], onesb[:], b3_sb[:], start=False, stop=True
                    )
                    nc.scalar.activation(
                        sout[:, j, :C], ps3[:, :C], AF.Copy,
                        scale=gsl[:, base // P + j, 0:1],
                    )
                nc.gpsimd.dma_scatter_add(
                    io["outpad"][:, :C], sout[:],
                    sidx16[:, base // 16 : (base + cs) // 16],
                    cs, cs, C, elem_step=CP,
                )

        # outpad[:T, :50] -> outbuf[:, :50] via SBUF; probs from acc.
        cp_sb = esb.tile([P, NT, C], F32, tag="cp_sb")
        nc.sync.dma_start(
            cp_sb[:], io["outpad"][:T, :C].rearrange("(i p) c -> p i c", p=P)
        )
        nc.sync.dma_start(
            io["outbuf"][:, :C].rearrange("(i p) c -> p i c", p=P), cp_sb[:]
        )
        acc = pools["acc"]
        nc.sync.dma_start(
            io["outbuf"][:, C : C + E].rearrange("(i p) e -> p i e", p=P),
            acc[:, :, C : C + E],
        )


def build(variant=None):
    variant = variant or VARIANT
    nc = bacc.Bacc(None, target_bir_lowering=False, debug=False)

    io = {}
    io["xT_f32"] = nc.dram_tensor("xT_f32", [D, T], F32, kind="ExternalInput")
    if variant == "dense":
        io["xT_bf16"] = nc.dram_tensor("xT_bf16", [D, T], BF16, kind="ExternalInput")
    else:
        io["x_bf"] = nc.dram_tensor("x_bf", [T, D], BF16, kind="ExternalInput")
        io["tri128"] = nc.dram_tensor("tri128", [P, P], F32, kind="ExternalInput")
        io["tri16s"] = nc.dram_tensor("tri16s", [16, 16], F32, kind="ExternalInput")
        io["idconst"] = nc.dram_tensor("idconst", [P, P], F32, kind="ExternalInput")
        io["capoff"] = nc.dram_tensor("capoff", [1, E], F32, kind="ExternalInput")
        io["trashv"] = nc.dram_tensor("trashv", [P, 1, 1], F32, kind="ExternalInput")
        io["trash16"] = nc.dram_tensor("trash16", [P, 1], F32, kind="ExternalInput")
        io["idxlist"] = nc.dram_tensor("idxlist", [NSLOT + P, CP], F32)
        io["dscratch"] = nc.dram_tensor("dscratch", [T * E], mybir.dt.int16)
        io["outpad"] = nc.dram_tensor("outpad", [NPAD, CP], F32)
    io["rW1"] = nc.dram_tensor("rW1", [D, RH], F32, kind="ExternalInput")
    io["rb1"] = nc.dram_tensor("rb1", [RH], F32, kind="ExternalInput")
    io["rW2"] = nc.dram_tensor("rW2", [RH, RH // 2], F32, kind="ExternalInput")
    io["rb2"] = nc.dram_tensor("rb2", [RH // 2], F32, kind="ExternalInput")
    io["rW3"] = nc.dram_tensor("rW3", [RH // 2, E], F32, kind="ExternalInput")
    io["rb3"] = nc.dram_tensor("rb3", [E], F32, kind="ExternalInput")
    for e in range(E):
        io[f"eW1_{e}"] = nc.dram_tensor(f"eW1_{e}", [D, H1], BF16, kind="ExternalInput")
        io[f"eb1_{e}"] = nc.dram_tensor(f"eb1_{e}", [H1], F32, kind="ExternalInput")
        io[f"eW2_{e}"] = nc.dram_tensor(f"eW2_{e}", [H1, H2], BF16, kind="ExternalInput")
        io[f"eb2_{e}"] = nc.dram_tensor(f"eb2_{e}", [H2], F32, kind="ExternalInput")
        io[f"eW3_{e}"] = nc.dram_tensor(f"eW3_{e}", [H2, C], BF16, kind="ExternalInput")
        io[f"eb3b_{e}"] = nc.dram_tensor(f"eb3b_{e}", [1, C], BF16, kind="ExternalInput")
    io["outbuf"] = nc.dram_tensor("outbuf", [T, CP], F32, kind="ExternalOutput")

    with tile.TileContext(nc) as tc:
        with tc.tile_pool(name="gates_pool", bufs=1) as gpool:
            acc = gpool.tile([P, NT, CP], F32, tag="acc", bufs=1)
            nc.vector.memset(acc[:], 0.0)
            pools = {"gates": gpool, "acc": acc}
            if variant == "dense":
                gates = _router(nc, tc, io, pools)
                _experts(nc, tc, io, pools, gates)
            else:
                from concourse import library_config
                nc.gpsimd.load_library(library_config.mlp)
                gates, mask = _router(nc, tc, io, pools, want_mask=True)
                gidx16, sidx16, gsl = _dispatch(nc, tc, io, pools, gates, mask)
                _experts_sparse(nc, tc, io, pools, gidx16, sidx16, gsl)

    nc.compile()
    return nc


def _get_built(caps=None):
    key = (VARIANT, tuple(caps) if caps else None)
    if key not in _built:
        if caps:
            set_caps(caps)
        _built[key] = build()
    return _built[key]


def make_in_maps(inputs, variant=None):
    """Shard FULL inputs into per-core in_maps."""
    variant = variant or VARIANT
    x = np.asarray(inputs["x"], np.float32)
    eW1 = np.asarray(inputs["eW1"], np.float32)
    eW2 = np.asarray(inputs["eW2"], np.float32)
    eW3 = np.asarray(inputs["eW3"], np.float32)
    eb1 = np.asarray(inputs["eb1"], np.float32)
    eb2 = np.asarray(inputs["eb2"], np.float32)
    eb3 = np.asarray(inputs["eb3"], np.float32)
    shared = {}
    for k in ("rW1", "rb1", "rW2", "rb2", "rW3", "rb3"):
        shared[k] = np.ascontiguousarray(np.asarray(inputs[k], np.float32))
    for e in range(E):
        shared[f"eW1_{e}"] = np.ascontiguousarray(eW1[e].astype(ml_dtypes.bfloat16))
        shared[f"eW2_{e}"] = np.ascontiguousarray(eW2[e].astype(ml_dtypes.bfloat16))
        shared[f"eW3_{e}"] = np.ascontiguousarray(eW3[e].astype(ml_dtypes.bfloat16))
        shared[f"eb1_{e}"] = np.ascontiguousarray(eb1[e])
        shared[f"eb2_{e}"] = np.ascontiguousarray(eb2[e])
        shared[f"eb3b_{e}"] = np.ascontiguousarray(
            eb3[e].astype(ml_dtypes.bfloat16)[None, :]
        )
    if variant == "sparse":
        p = np.arange(P, dtype=np.float32)
        shared["tri128"] = (
            (np.arange(P)[:, None] <= np.arange(P)[None, :]).astype(np.float32)
        )
        shared["tri16s"] = (
            (np.arange(16)[:, None] < np.arange(16)[None, :]).astype(np.float32)
        )
        shared["idconst"] = np.ascontiguousarray(
            1.0 + (np.arange(P)[None, :] % 16) * 128 + p[:, None]
        ).astype(np.float32)
        shared["capoff"] = np.asarray(OFFS, np.float32)[None, :]
        shared["trashv"] = (NSLOT + p).astype(np.float32)[:, None, None]
        shared["trash16"] = (T + np.arange(P, dtype=np.float32) % 16)[:, None]

    in_maps = []
    for c in range(NCORES):
        xs = x[c * T : (c + 1) * T]
        xT = np.ascontiguousarray(xs.T)
        m = dict(shared)
        m["xT_f32"] = xT
        if variant == "dense":
            m["xT_bf16"] = xT.astype(ml_dtypes.bfloat16)
        else:
            m["x_bf"] = xs.astype(ml_dtypes.bfloat16)
        in_maps.append(m)
    return in_maps


def _run(inputs, variant):
    global VARIANT
    VARIANT = variant
    caps = compute_caps(inputs) if variant == "sparse" else None
    nc = _get_built(caps)
    in_maps = make_in_maps(inputs)
    res = run_bass_kernel_spmd(nc, in_maps, core_ids=list(range(NCORES)))
    buf = np.concatenate([res.results[c]["outbuf"] for c in range(NCORES)], axis=0)
    return np.ascontiguousarray(buf[:, :C]), np.ascontiguousarray(buf[:, C : C + E])


def kernel(**inputs):
    assert int(inputs.get("top_k", 2)) == 2
    if VARIANT == "sparse":
        try:
            return _run(inputs, "sparse")
        except Exception:
            import traceback
            traceback.print_exc()
            print("kernel: sparse variant failed; falling back to dense")
    return _run(inputs, "dense")
